# revision 7
# baseline (speedup 1.0000x reference)
"""Trainium2 Bass kernel for nn_Attention_80779744903968.

Reference computation (B=32, T=512, S=1024, H=1024):
    z      = q @ W_in.T                  [B,T,H]
    scores = z @ enc_b.T                 [B,T,S]   (enc input is [S,B,H])
    p      = softmax(scores, axis=-1)    (the scores==0 -> -inf fill is a
                                          numerical no-op: row maxes are ~120,
                                          exp(0-max) == 0 in fp32)
    c      = p @ enc_b                   [B,T,H]
    out    = tanh([c, q] @ W_out.T + b)  [B,T,H]

Sharding: data-parallel over B across 8 cores (4 batches per core).
W_in / W_out replicated.

Precision: z and scores are computed as an fp16 hi/lo split -- fp16 main
pass (fp32 PSUM) plus fp8(e4m3) DoubleRow correction passes
(xh*yl + xl*yh).  Downstream (p, enc, c, q, W_out) is plain fp16.

Scale matching: operands are pre-scaled by powers of two so the main and
correction PSUM tiles of the scores matmul land at the same scale (2^18),
letting one tensor_tensor_reduce do {evict + corr-add + row-max} per chunk
and the Exp activation (scale=2^-18, accum_out) produce the softmax sum
for free:
    wh  : W_in.T hi  * 2^13 (f16)   z main psum  = z*2^13
    wl8 : lo * 2^13, wh8: hi * 2^4, qh8: q*1, ql8: lo * 2^9
                                    z corr accumulates into the same psum
    zh  : f16(z*2^13) on device; zl8 = (comb - zh) fp8 (scale 2^13);
    zh8 = zh*2^-8 fp8 (scale 2^5)
    eh  : enc.T hi * 2^5 (f16), eh8: hi * 2^5 fp8, el8: lo * 2^13 fp8
    scores main = zh*eh = s*2^18;  corr = zl8*eh8 + zh8*el8 = s*2^18

Engine split: PE matmuls; DVE does fused psum evictions (STT / TTR) +
small stats; Scalar (Activation) does exp/p-normalize/casts/copies/tanh;
GpSimd issues bulk input DMA descriptors; Sync issues weights/q + output
DMAs.  ~64 dummy PE transposes at kernel start warm the HAM clock gate
while the first DMAs land.
"""
import os
import sys

import numpy as np

sys.path.insert(0, "/opt/trn_rl_repo")

import ml_dtypes  # noqa: E402

import concourse.bass as bass  # noqa: E402
import concourse.tile as tile  # noqa: E402
from concourse import bacc, mybir  # noqa: E402
from concourse.bass_utils import run_bass_kernel_spmd  # noqa: E402
from concourse.masks import make_identity  # noqa: E402

B, T, S, H = 32, 512, 1024, 1024
NCORES = 8
BL = B // NCORES  # batches per core
HT = H // 128     # h/i/k tiles per 1024
TT = T // 128     # t tiles
ST = S // 128     # s tiles
F16 = mybir.dt.float16
F32 = mybir.dt.float32
F8 = mybir.dt.float8e4
DR = mybir.MatmulPerfMode.DoubleRow
Alu = mybir.AluOpType
Act = mybir.ActivationFunctionType

N_WARM = 64           # HAM warm-up dummy transposes
SC_SCALE = 2.0 ** -18  # scores psum scale -> logits
NEG_INIT = -3.0e38

_CACHE = {}


def _build(has_bias):
    nc = bacc.Bacc("TRN2", target_bir_lowering=False, debug=False,
                   num_devices=NCORES)

    def din(name, shape, dt=F16):
        return nc.dram_tensor(name, shape, dt, kind="ExternalInput").ap()

    qh_d = din("qh", [BL, H, T])
    eh_d = din("eh", [BL, H, S])
    en_d = din("en", [BL, S, H])
    wh_d = din("wh", [H, H])
    wo_d = din("wo", [2 * H, H])
    ql8_d = din("ql8", [BL, H, T], F8)
    el8_d = din("el8", [BL, H, S], F8)
    qh8_d = din("qh8", [BL, H, T], F8)
    eh8_d = din("eh8", [BL, H, S], F8)
    wh8_d = din("wh8", [H, H], F8)
    wl8_d = din("wl8", [H, H], F8)
    if has_bias:
        bias_d = din("bias", [128, H], F32)
    out_d = nc.dram_tensor("out", [BL, T, H], F32, kind="ExternalOutput").ap()

    with tile.TileContext(nc) as tc:
        with (
            tc.tile_pool(name="weights", bufs=1) as wp,
            tc.tile_pool(name="qin", bufs=2) as qp,
            tc.tile_pool(name="ein", bufs=1) as ep,
            tc.tile_pool(name="enin", bufs=1) as enp,
            tc.tile_pool(name="zbuf", bufs=1) as zp,
            tc.tile_pool(name="scores", bufs=2) as scp,
            tc.tile_pool(name="pbuf", bufs=3) as pp,
            tc.tile_pool(name="ptbuf", bufs=1) as ptp,
            tc.tile_pool(name="ctbuf", bufs=1) as ctp,
            tc.tile_pool(name="ostage", bufs=2) as op,
            tc.tile_pool(name="stats", bufs=2) as stp,
            tc.tile_pool(name="psmm", bufs=6, space="PSUM") as psmm,
            tc.tile_pool(name="pstr", bufs=2, space="PSUM") as pstr,
        ):
            # --- identity first: gpsimd builds it, PE dummies warm HAM ---
            ident = wp.tile([128, 128], F16)
            make_identity(nc, ident[:])

            # --- weight + b0 q DMA issues on Sync, ordered by first use ---
            wh_t = wp.tile([128, HT, H], F16)
            wh_r = wh_d.rearrange("(ht p) i -> p ht i", p=128)
            qh_first = qp.tile([128, HT, T], F16, tag="qh")
            qh_r0 = qh_d[0].rearrange("(ht p) t -> p ht t", p=128)
            nc.sync.dma_start(wh_t[:, 0:4, :], wh_r[:, 0:4, :])
            nc.sync.dma_start(qh_first[:, 0:4, :], qh_r0[:, 0:4, :])
            nc.sync.dma_start(wh_t[:, 4:8, :], wh_r[:, 4:8, :])
            nc.sync.dma_start(qh_first[:, 4:8, :], qh_r0[:, 4:8, :])
            wl8_t = wp.tile([128, HT, H], F8)
            nc.sync.dma_start(
                wl8_t[:], wl8_d.rearrange("(ht p) i -> p ht i", p=128))
            qh8_first = qp.tile([128, HT, T], F8, tag="qh8", bufs=1)
            nc.sync.dma_start(
                qh8_first[:], qh8_d[0].rearrange("(ht p) t -> p ht t", p=128))
            wh8_t = wp.tile([128, HT, H], F8)
            nc.sync.dma_start(
                wh8_t[:], wh8_d.rearrange("(ht p) i -> p ht i", p=128))
            ql8_first = qp.tile([128, HT, T], F8, tag="ql8", bufs=1)
            nc.sync.dma_start(
                ql8_first[:], ql8_d[0].rearrange("(ht p) t -> p ht t", p=128))

            # --- b0 bulk inputs: eh-side on Vector queue, rest on GpSimd ---
            eh_first = ep.tile([128, HT, S], F16, tag="eh", bufs=2)
            nc.scalar.dma_start(
                eh_first[:], eh_d[0].rearrange("(it p) s -> p it s", p=128))
            eh8_first = ep.tile([128, HT, S], F8, tag="eh8", bufs=1)
            nc.scalar.dma_start(
                eh8_first[:], eh8_d[0].rearrange("(it p) s -> p it s", p=128))
            el8_first = ep.tile([128, HT, S], F8, tag="el8", bufs=1)
            nc.scalar.dma_start(
                el8_first[:], el8_d[0].rearrange("(it p) s -> p it s", p=128))
            en_first = enp.tile([128, ST, H], F16, tag="en")
            nc.gpsimd.dma_start(
                en_first[:], en_d[0].rearrange("(st p) k -> p st k", p=128))
            wo_t = wp.tile([128, 2 * HT, H], F16)
            wo_r = wo_d.rearrange("(kt p) h -> p kt h", p=128)
            for kc in range(4):
                nc.gpsimd.dma_start(
                    wo_t[:, 4 * kc:4 * kc + 4, :], wo_r[:, 4 * kc:4 * kc + 4, :])
            if has_bias:
                bias_t = wp.tile([128, H], F32)
                nc.gpsimd.dma_start(bias_t[:], bias_d)

            # --- HAM warm-up: dummy PE transposes while DMA streams in ---
            for w in range(N_WARM):
                wtile = pstr.tile([128, 128], F16, tag="tr", name=f"warm{w}")
                nc.tensor.transpose(wtile[:], ident[:], ident[:])

            for b in range(BL):
                if b == 0:
                    qh_t, qh8_t, ql8_t = qh_first, qh8_first, ql8_first
                    eh_t, eh8_t, el8_t = eh_first, eh8_first, el8_first
                    en_t = en_first
                else:
                    qh_t = qp.tile([128, HT, T], F16, tag="qh")
                    nc.gpsimd.dma_start(
                        qh_t[:], qh_d[b].rearrange("(ht p) t -> p ht t", p=128))
                    eh_t = ep.tile([128, HT, S], F16, tag="eh", bufs=2)
                    nc.gpsimd.dma_start(
                        eh_t[:], eh_d[b].rearrange("(it p) s -> p it s", p=128))
                    qh8_t = qp.tile([128, HT, T], F8, tag="qh8", bufs=1)
                    nc.gpsimd.dma_start(
                        qh8_t[:], qh8_d[b].rearrange("(ht p) t -> p ht t", p=128))
                    ql8_t = qp.tile([128, HT, T], F8, tag="ql8", bufs=1)
                    nc.gpsimd.dma_start(
                        ql8_t[:], ql8_d[b].rearrange("(ht p) t -> p ht t", p=128))
                    eh8_t = ep.tile([128, HT, S], F8, tag="eh8", bufs=1)
                    nc.gpsimd.dma_start(
                        eh8_t[:], eh8_d[b].rearrange("(it p) s -> p it s", p=128))
                    el8_t = ep.tile([128, HT, S], F8, tag="el8", bufs=1)
                    nc.gpsimd.dma_start(
                        el8_t[:], el8_d[b].rearrange("(it p) s -> p it s", p=128))
                    en_t = enp.tile([128, ST, H], F16, tag="en")
                    nc.gpsimd.dma_start(
                        en_t[:], en_d[b].rearrange("(st p) k -> p st k", p=128))

                # --- z: main (f16) + corr (fp8 DR) accumulate into ONE
                # psum group per i-tile, all at scale 2^13 ---
                zh_t = zp.tile([128, HT, T], F16, tag="zh")
                zh8_t = zp.tile([128, HT, T], F8, tag="zh8")
                zl8_t = zp.tile([128, HT, T], F8, tag="zl8")

                def z_main(it):
                    zps = psmm.tile([128, T], F32, tag="mm", name=f"zps{it}")
                    for ht in range(HT):
                        nc.tensor.matmul(
                            zps[:],
                            wh_t[:, ht, it * 128:(it + 1) * 128],
                            qh_t[:, ht, :],
                            start=(ht == 0), stop=False)
                    return zps

                def z_corr_evict(it, zps):
                    j = 0
                    for lhs, rhs in ((wl8_t, qh8_t), (wh8_t, ql8_t)):
                        for k in range(HT // 2):
                            nc.tensor.matmul(
                                zps[:],
                                lhs[:, 2 * k:2 * k + 2, it * 128:(it + 1) * 128],
                                rhs[:, 2 * k:2 * k + 2, :],
                                start=False, stop=(j == HT - 1),
                                perf_mode=DR, skip_group_check=True)
                            j += 1
                    nc.scalar.copy(zh_t[:, it, :], zps[:])
                    nc.vector.scalar_tensor_tensor(
                        out=zl8_t[:, it, :], in0=zh_t[:, it, :], scalar=-1.0,
                        in1=zps[:], op0=Alu.mult, op1=Alu.add)
                    nc.vector.tensor_scalar_mul(
                        zh8_t[:, it, :], zh_t[:, it, :], 2.0 ** -8)

                # two half-passes: 4 mains, then their corrs (b0: lets the
                # fp8 weight DMAs land while the f16 mains stream)
                for half in range(2):
                    its = range(half * 4, half * 4 + 4)
                    zpss = {it: z_main(it) for it in its}
                    for it in its:
                        z_corr_evict(it, zpss[it])

                # --- scores + softmax (+ interleaved transposes of tt-1) ---
                p_tiles = {}
                pt_t = ptp.tile([128, ST, T], F16, tag="pt")
                tr_pending = []

                def emit_tr(n):
                    for _ in range(min(n, len(tr_pending))):
                        tt0, st = tr_pending.pop(0)
                        tps = pstr.tile([128, 128], F16, tag="tr")
                        nc.tensor.transpose(
                            tps[:],
                            p_tiles[tt0][:, st * 128:(st + 1) * 128],
                            ident[:])
                        if st % 2 == 0:
                            nc.vector.tensor_copy(
                                pt_t[:, st, tt0 * 128:(tt0 + 1) * 128], tps[:])
                        else:
                            nc.scalar.copy(
                                pt_t[:, st, tt0 * 128:(tt0 + 1) * 128], tps[:])

                for tt in range(TT):
                    sc_t = scp.tile([128, S], F32, tag="sc")
                    for sc in range(2):
                        sps = psmm.tile([128, 512], F32, tag="mm")
                        for it in range(HT):
                            nc.tensor.matmul(
                                sps[:],
                                zh_t[:, it, tt * 128:(tt + 1) * 128],
                                eh_t[:, it, sc * 512:(sc + 1) * 512],
                                start=(it == 0), stop=False)
                        if sc == 1:
                            emit_tr(2)
                        j = 0
                        for lhs, rhs in ((zl8_t, eh8_t), (zh8_t, el8_t)):
                            for k in range(HT // 2):
                                nc.tensor.matmul(
                                    sps[:],
                                    lhs[:, 2 * k:2 * k + 2,
                                        tt * 128:(tt + 1) * 128],
                                    rhs[:, 2 * k:2 * k + 2,
                                        sc * 512:(sc + 1) * 512],
                                    start=False, stop=(j == HT - 1),
                                    perf_mode=DR, skip_group_check=True)
                                j += 1
                        emit_tr(2)
                        nc.vector.tensor_copy(
                            sc_t[:, sc * 512:(sc + 1) * 512], sps[:])
                    # softmax over free dim (s); psum scale folded into exp
                    negmax = stp.tile([128, 1], F32, tag="nm")
                    nc.vector.reduce_max(out=negmax[:], in_=sc_t[:],
                                         axis=mybir.AxisListType.X, negate=True)
                    nc.vector.tensor_scalar_mul(negmax[:], negmax[:], SC_SCALE)
                    p_t = pp.tile([128, S], F16, tag="p")
                    ssum = stp.tile([128, 1], F32, tag="ss")
                    nc.scalar.activation(
                        out=p_t[:], in_=sc_t[:], func=Act.Exp,
                        bias=negmax[:], scale=SC_SCALE, accum_out=ssum[:])
                    rsum = stp.tile([128, 1], F32, tag="rs")
                    nc.vector.reciprocal(rsum[:], ssum[:])
                    nc.scalar.mul(p_t[:], p_t[:], rsum[:])
                    p_tiles[tt] = p_t
                    tr_pending.extend((tt, st) for st in range(ST))
                    if tt > 0:
                        emit_tr(2)

                # remaining transposes (tt3); c consumes st in order
                emit_tr(len(tr_pending))

                # --- cT = enc_nat.T @ pT -> [k, t] f16 ---
                ct_t = ctp.tile([128, HT, T], F16, tag="ct")
                for kt in range(HT):
                    cps = psmm.tile([128, T], F32, tag="mm")
                    for st in range(ST):
                        nc.tensor.matmul(
                            cps[:],
                            en_t[:, st, kt * 128:(kt + 1) * 128],
                            pt_t[:, st, :],
                            start=(st == 0), stop=(st == ST - 1))
                    nc.scalar.copy(ct_t[:, kt, :], cps[:])

                # --- out = tanh(cT.T @ WcT + qT.T @ WqT [+ b]) ---
                for tt in range(TT):
                    for hc in range(2):
                        ops = psmm.tile([128, 512], F32, tag="mm")
                        # q-part first: gives tail cT evictions extra slack
                        for ht in range(HT):
                            nc.tensor.matmul(
                                ops[:],
                                qh_t[:, ht, tt * 128:(tt + 1) * 128],
                                wo_t[:, HT + ht, hc * 512:(hc + 1) * 512],
                                start=(ht == 0), stop=False)
                        for kt in range(HT):
                            nc.tensor.matmul(
                                ops[:],
                                ct_t[:, kt, tt * 128:(tt + 1) * 128],
                                wo_t[:, kt, hc * 512:(hc + 1) * 512],
                                start=False, stop=(kt == HT - 1))
                        ost = op.tile([128, 512], F32, tag="os")
                        if has_bias:
                            nc.vector.tensor_add(
                                ost[:], ops[:],
                                bias_t[:, hc * 512:(hc + 1) * 512])
                            nc.scalar.activation(
                                out=ost[:], in_=ost[:], func=Act.Tanh)
                        else:
                            nc.scalar.activation(
                                out=ost[:], in_=ops[:], func=Act.Tanh)
                        nc.sync.dma_start(
                            out_d[b, tt * 128:(tt + 1) * 128,
                                  hc * 512:(hc + 1) * 512],
                            ost[:])

    nc.compile()
    return nc


def _get_nc(has_bias):
    key = ("nc", has_bias)
    if key not in _CACHE:
        _CACHE[key] = _build(has_bias)
    return _CACHE[key]


def _split16(x):
    hi = x.astype(np.float16)
    lo = (x - hi.astype(np.float32)).astype(np.float32)
    return hi, lo


def _f8(x, scale):
    return (np.asarray(x, np.float32) * np.float32(scale)).astype(
        ml_dtypes.float8_e4m3)


def kernel(query, encoder_outputs, src_lengths, W_in, W_out, b_out):
    query = np.asarray(query, np.float32)
    enc = np.asarray(encoder_outputs, np.float32)
    W_in = np.asarray(W_in, np.float32)
    W_out = np.asarray(W_out, np.float32)
    b_out = np.asarray(b_out, np.float32)

    # host-side layout prep (transposes + fp16 hi/lo splits + 2^k scaling)
    qT = np.ascontiguousarray(query.transpose(0, 2, 1))        # [B, H, T]
    qh, ql = _split16(qT)
    encT = np.ascontiguousarray(enc.transpose(1, 2, 0))        # [B, H, S]
    ehf, el = _split16(encT)
    eh = (ehf.astype(np.float32) * 2.0 ** 5).astype(np.float16)
    en = np.ascontiguousarray(enc.transpose(1, 0, 2)).astype(np.float16)
    whf, wlf = _split16(np.ascontiguousarray(W_in.T))          # [H(h), H(i)]
    wh = (whf.astype(np.float32) * 2.0 ** 13).astype(np.float16)
    wo = np.ascontiguousarray(W_out.T).astype(np.float16)      # [2H, H]

    has_bias = bool(np.any(b_out))
    common = {
        "wh": wh, "wo": wo,
        "wh8": _f8(whf.astype(np.float32), 2.0 ** 4),
        "wl8": _f8(wlf, 2.0 ** 13),
    }
    if has_bias:
        common["bias"] = np.ascontiguousarray(
            np.broadcast_to(b_out[None, :], (128, H)), np.float32)

    in_maps = []
    for c in range(NCORES):
        sl = slice(c * BL, (c + 1) * BL)
        m = {
            "qh": np.ascontiguousarray(qh[sl]),
            "eh": np.ascontiguousarray(eh[sl]),
            "en": np.ascontiguousarray(en[sl]),
            "qh8": _f8(qh[sl].astype(np.float32), 1.0),
            "ql8": _f8(ql[sl], 2.0 ** 9),
            "eh8": _f8(ehf[sl].astype(np.float32), 2.0 ** 5),
            "el8": _f8(el[sl], 2.0 ** 13),
            **common,
        }
        in_maps.append(m)

    nc = _get_nc(has_bias)
    trace = bool(int(os.environ.get("KERNEL_TRACE", "0")))
    res = run_bass_kernel_spmd(nc, in_maps, core_ids=list(range(NCORES)),
                               trace=trace)
    if trace:
        _CACHE["last_exec_time_ns"] = res.exec_time_ns
        _CACHE["last_results"] = res
    out = np.concatenate([r["out"] for r in res.results], axis=0)
    return out


# revision 13
# speedup vs baseline: 1.0471x; 1.0471x over previous
"""Trainium2 Bass kernel for nn_Attention_80779744903968.

Reference computation (B=32, T=512, S=1024, H=1024):
    z      = q @ W_in.T                  [B,T,H]
    scores = z @ enc_b.T                 [B,T,S]   (enc input is [S,B,H])
    p      = softmax(scores, axis=-1)    (the scores==0 -> -inf fill is a
                                          numerical no-op: row maxes are ~120,
                                          exp(0-max) == 0 in fp32)
    c      = p @ enc_b                   [B,T,H]
    out    = tanh([c, q] @ W_out.T + b)  [B,T,H]

Sharding: data-parallel over B across 8 cores (4 batches per core).
W_in / W_out replicated.

Precision: z and scores are computed as an fp16 hi/lo split -- fp16 main
pass (fp32 PSUM) plus fp8(e4m3) DoubleRow correction passes
(xh*yl + xl*yh).  Downstream (p, enc, c, q, W_out) is plain fp16.

Scale matching: operands are pre-scaled by powers of two so the main and
correction PSUM tiles of the scores matmul land at the same scale (2^18),
letting one tensor_tensor_reduce do {evict + corr-add + row-max} per chunk
and the Exp activation (scale=2^-18, accum_out) produce the softmax sum
for free:
    wh  : W_in.T hi  * 2^13 (f16)   z main psum  = z*2^13
    wl8 : lo * 2^13, wh8: hi * 2^4, qh8: q*1, ql8: lo * 2^9
                                    z corr accumulates into the same psum
    zh  : f16(z*2^13) on device; zl8 = (comb - zh) fp8 (scale 2^13);
    zh8 = zh*2^-8 fp8 (scale 2^5)
    eh  : enc.T hi * 2^5 (f16), eh8: hi * 2^5 fp8, el8: lo * 2^13 fp8
    scores main = zh*eh = s*2^18;  corr = zl8*eh8 + zh8*el8 = s*2^18

Engine split: PE matmuls; DVE does fused psum evictions (STT / TTR) +
small stats; Scalar (Activation) does exp/p-normalize/casts/copies/tanh;
GpSimd issues bulk input DMA descriptors; Sync issues weights/q + output
DMAs.  ~64 dummy PE transposes at kernel start warm the HAM clock gate
while the first DMAs land.
"""
import os
import sys

import numpy as np

sys.path.insert(0, "/opt/trn_rl_repo")

import ml_dtypes  # noqa: E402

import concourse.bass as bass  # noqa: E402
import concourse.tile as tile  # noqa: E402
from concourse import bacc, mybir  # noqa: E402
from concourse.bass_utils import run_bass_kernel_spmd  # noqa: E402
from concourse.masks import make_identity  # noqa: E402

B, T, S, H = 32, 512, 1024, 1024
NCORES = 8
BL = B // NCORES  # batches per core
HT = H // 128     # h/i/k tiles per 1024
TT = T // 128     # t tiles
ST = S // 128     # s tiles
F16 = mybir.dt.float16
F32 = mybir.dt.float32
F8 = mybir.dt.float8e4
DR = mybir.MatmulPerfMode.DoubleRow
Alu = mybir.AluOpType
Act = mybir.ActivationFunctionType

N_WARM = 64           # HAM warm-up dummy transposes
SC_SCALE = 2.0 ** -18  # scores psum scale -> logits
NEG_INIT = -3.0e38

_CACHE = {}


def _build(has_bias):
    nc = bacc.Bacc("TRN2", target_bir_lowering=False, debug=False,
                   num_devices=NCORES)

    def din(name, shape, dt=F16):
        return nc.dram_tensor(name, shape, dt, kind="ExternalInput").ap()

    qh_d = din("qh", [BL, H, T])
    eh_d = din("eh", [BL, H, S])
    en_d = din("en", [BL, S, H])
    wh_d = din("wh", [H, H])
    wo_d = din("wo", [2 * H, H])
    ql8_d = din("ql8", [BL, H, T], F8)
    el8_d = din("el8", [BL, H, S], F8)
    qh8_d = din("qh8", [BL, H, T], F8)
    eh8_d = din("eh8", [BL, H, S], F8)
    wh8_d = din("wh8", [H, H], F8)
    wl8_d = din("wl8", [H, H], F8)
    if has_bias:
        bias_d = din("bias", [128, H], F32)
    out_d = nc.dram_tensor("out", [BL, T, H], F32, kind="ExternalOutput").ap()

    with tile.TileContext(nc) as tc:
        with (
            tc.tile_pool(name="weights", bufs=1) as wp,
            tc.tile_pool(name="qin", bufs=2) as qp,
            tc.tile_pool(name="ein", bufs=1) as ep,
            tc.tile_pool(name="enin", bufs=1) as enp,
            tc.tile_pool(name="zbuf", bufs=1) as zp,
            tc.tile_pool(name="scores", bufs=2) as scp,
            tc.tile_pool(name="pbuf", bufs=3) as pp,
            tc.tile_pool(name="ptbuf", bufs=1) as ptp,
            tc.tile_pool(name="ctbuf", bufs=1) as ctp,
            tc.tile_pool(name="ostage", bufs=2) as op,
            tc.tile_pool(name="stats", bufs=2) as stp,
            tc.tile_pool(name="psmm", bufs=6, space="PSUM") as psmm,
            tc.tile_pool(name="pstr", bufs=2, space="PSUM") as pstr,
        ):
            # --- identity first: gpsimd builds it, PE dummies warm HAM ---
            ident = wp.tile([128, 128], F16)
            make_identity(nc, ident[:])

            # --- weight + b0 q DMA issues on Sync, ordered by first use ---
            wh_t = wp.tile([128, HT, H], F16)
            wh_r = wh_d.rearrange("(ht p) i -> p ht i", p=128)
            qh_first = qp.tile([128, HT, T], F16, tag="qh")
            qh_r0 = qh_d[0].rearrange("(ht p) t -> p ht t", p=128)
            nc.sync.dma_start(wh_t[:, 0:4, :], wh_r[:, 0:4, :])
            nc.sync.dma_start(qh_first[:, 0:4, :], qh_r0[:, 0:4, :])
            nc.sync.dma_start(wh_t[:, 4:8, :], wh_r[:, 4:8, :])
            nc.sync.dma_start(qh_first[:, 4:8, :], qh_r0[:, 4:8, :])
            wl8_t = wp.tile([128, HT, H], F8)
            nc.sync.dma_start(
                wl8_t[:], wl8_d.rearrange("(ht p) i -> p ht i", p=128))
            qh8_first = qp.tile([128, HT, T], F8, tag="qh8", bufs=1)
            nc.sync.dma_start(
                qh8_first[:], qh8_d[0].rearrange("(ht p) t -> p ht t", p=128))
            wh8_t = wp.tile([128, HT, H], F8)
            nc.sync.dma_start(
                wh8_t[:], wh8_d.rearrange("(ht p) i -> p ht i", p=128))
            ql8_first = qp.tile([128, HT, T], F8, tag="ql8", bufs=1)
            nc.sync.dma_start(
                ql8_first[:], ql8_d[0].rearrange("(ht p) t -> p ht t", p=128))

            # --- b0 bulk inputs: eh-side on Vector queue, rest on GpSimd ---
            eh_first = ep.tile([128, HT, S], F16, tag="eh", bufs=2)
            nc.scalar.dma_start(
                eh_first[:], eh_d[0].rearrange("(it p) s -> p it s", p=128))
            eh8_first = ep.tile([128, HT, S], F8, tag="eh8", bufs=1)
            nc.scalar.dma_start(
                eh8_first[:], eh8_d[0].rearrange("(it p) s -> p it s", p=128))
            el8_first = ep.tile([128, HT, S], F8, tag="el8", bufs=1)
            nc.scalar.dma_start(
                el8_first[:], el8_d[0].rearrange("(it p) s -> p it s", p=128))
            en_first = enp.tile([128, ST, H], F16, tag="en")
            nc.gpsimd.dma_start(
                en_first[:], en_d[0].rearrange("(st p) k -> p st k", p=128))
            wo_t = wp.tile([128, 2 * HT, H], F16)
            wo_r = wo_d.rearrange("(kt p) h -> p kt h", p=128)
            for kc in range(4):
                nc.gpsimd.dma_start(
                    wo_t[:, 4 * kc:4 * kc + 4, :], wo_r[:, 4 * kc:4 * kc + 4, :])
            if has_bias:
                bias_t = wp.tile([128, H], F32)
                nc.gpsimd.dma_start(bias_t[:], bias_d)

            # --- HAM warm-up: dummy PE transposes while DMA streams in ---
            for w in range(N_WARM):
                wtile = pstr.tile([128, 128], F16, tag="tr", name=f"warm{w}")
                nc.tensor.transpose(wtile[:], ident[:], ident[:])

            zh_prev = None
            for b in range(BL):
                if b == 0:
                    qh_t, qh8_t, ql8_t = qh_first, qh8_first, ql8_first
                    eh_t, eh8_t, el8_t = eh_first, eh8_first, el8_first
                    en_t = en_first
                else:
                    # gate batch-b prefetch behind z(b-1): keeps the DMA
                    # queues clear for the previous batch's critical loads
                    gate = stp.tile([128, 1], F16, tag="gate")
                    nc.gpsimd.tensor_copy(gate[:], zh_prev[:, 7, 511:512])
                    qh_t = qp.tile([128, HT, T], F16, tag="qh")
                    nc.gpsimd.dma_start(
                        qh_t[:], qh_d[b].rearrange("(ht p) t -> p ht t", p=128))
                    eh_t = ep.tile([128, HT, S], F16, tag="eh", bufs=2)
                    nc.gpsimd.dma_start(
                        eh_t[:], eh_d[b].rearrange("(it p) s -> p it s", p=128))
                    qh8_t = qp.tile([128, HT, T], F8, tag="qh8", bufs=1)
                    nc.gpsimd.dma_start(
                        qh8_t[:], qh8_d[b].rearrange("(ht p) t -> p ht t", p=128))
                    ql8_t = qp.tile([128, HT, T], F8, tag="ql8", bufs=1)
                    nc.gpsimd.dma_start(
                        ql8_t[:], ql8_d[b].rearrange("(ht p) t -> p ht t", p=128))
                    eh8_t = ep.tile([128, HT, S], F8, tag="eh8", bufs=1)
                    nc.gpsimd.dma_start(
                        eh8_t[:], eh8_d[b].rearrange("(it p) s -> p it s", p=128))
                    el8_t = ep.tile([128, HT, S], F8, tag="el8", bufs=1)
                    nc.gpsimd.dma_start(
                        el8_t[:], el8_d[b].rearrange("(it p) s -> p it s", p=128))
                    en_t = enp.tile([128, ST, H], F16, tag="en")
                    nc.gpsimd.dma_start(
                        en_t[:], en_d[b].rearrange("(st p) k -> p st k", p=128))

                # --- z: main (f16) + corr (fp8 DR) accumulate into ONE
                # psum group per i-tile, all at scale 2^13 ---
                zh_t = zp.tile([128, HT, T], F16, tag="zh")
                zh8_t = zp.tile([128, HT, T], F8, tag="zh8")
                zl8_t = zp.tile([128, HT, T], F8, tag="zl8")

                def z_main(it):
                    zps = psmm.tile([128, T], F32, tag="mm", name=f"zps{it}")
                    for ht in range(HT):
                        nc.tensor.matmul(
                            zps[:],
                            wh_t[:, ht, it * 128:(it + 1) * 128],
                            qh_t[:, ht, :],
                            start=(ht == 0), stop=False)
                    return zps

                def z_corr_evict(it, zps):
                    j = 0
                    for lhs, rhs in ((wl8_t, qh8_t), (wh8_t, ql8_t)):
                        for k in range(HT // 2):
                            nc.tensor.matmul(
                                zps[:],
                                lhs[:, 2 * k:2 * k + 2, it * 128:(it + 1) * 128],
                                rhs[:, 2 * k:2 * k + 2, :],
                                start=False, stop=(j == HT - 1),
                                perf_mode=DR, skip_group_check=True)
                            j += 1
                    nc.scalar.copy(zh_t[:, it, :], zps[:])
                    nc.vector.scalar_tensor_tensor(
                        out=zl8_t[:, it, :], in0=zh_t[:, it, :], scalar=-1.0,
                        in1=zps[:], op0=Alu.mult, op1=Alu.add)
                    nc.vector.tensor_scalar_mul(
                        zh8_t[:, it, :], zh_t[:, it, :], 2.0 ** -8)

                # two half-passes: 4 mains, then their corrs (b0: lets the
                # fp8 weight DMAs land while the f16 mains stream)
                for half in range(2):
                    its = range(half * 4, half * 4 + 4)
                    zpss = {it: z_main(it) for it in its}
                    for it in its:
                        z_corr_evict(it, zpss[it])
                zh_prev = zh_t

                # --- scores + softmax; transposes of tt run two tts later
                # (the softmax chain is ~4-5us deep) ---
                p_tiles = {}
                pt_t = ptp.tile([128, ST, T], F16, tag="pt")
                tr_pending = []

                def emit_tr(n):
                    for _ in range(min(n, len(tr_pending))):
                        tt0, st = tr_pending.pop(0)
                        tps = pstr.tile([128, 128], F16, tag="tr")
                        nc.tensor.transpose(
                            tps[:],
                            p_tiles[tt0][:, st * 128:(st + 1) * 128],
                            ident[:])
                        if st % 2 == 0:
                            nc.vector.tensor_copy(
                                pt_t[:, st, tt0 * 128:(tt0 + 1) * 128], tps[:])
                        else:
                            nc.scalar.copy(
                                pt_t[:, st, tt0 * 128:(tt0 + 1) * 128], tps[:])

                for tt in range(TT):
                    if tt >= 2:
                        tr_pending.extend((tt - 2, st) for st in range(ST))
                    sc_t = scp.tile([128, S], F32, tag="sc")
                    cmax = {}
                    for sc in range(2):
                        sps = psmm.tile([128, 512], F32, tag="mm")
                        for it in range(HT):
                            nc.tensor.matmul(
                                sps[:],
                                zh_t[:, it, tt * 128:(tt + 1) * 128],
                                eh_t[:, it, sc * 512:(sc + 1) * 512],
                                start=(it == 0), stop=False)
                        emit_tr(2)
                        j = 0
                        for lhs, rhs in ((zl8_t, eh8_t), (zh8_t, el8_t)):
                            for k in range(HT // 2):
                                nc.tensor.matmul(
                                    sps[:],
                                    lhs[:, 2 * k:2 * k + 2,
                                        tt * 128:(tt + 1) * 128],
                                    rhs[:, 2 * k:2 * k + 2,
                                        sc * 512:(sc + 1) * 512],
                                    start=False, stop=(j == HT - 1),
                                    perf_mode=DR, skip_group_check=True)
                                j += 1
                        emit_tr(2)
                        nc.vector.tensor_copy(
                            sc_t[:, sc * 512:(sc + 1) * 512], sps[:])
                        # per-chunk max: chunk0's reduce hides under chunk1
                        cm = stp.tile([128, 1], F32, tag=f"cm{sc}")
                        nc.vector.reduce_max(
                            out=cm[:], in_=sc_t[:, sc * 512:(sc + 1) * 512],
                            axis=mybir.AxisListType.X)
                        cmax[sc] = cm
                    # softmax over free dim (s); psum scale folded into exp
                    negmax = stp.tile([128, 1], F32, tag="nm")
                    nc.vector.tensor_tensor(
                        out=negmax[:], in0=cmax[0][:], in1=cmax[1][:],
                        op=Alu.max)
                    nc.vector.tensor_scalar_mul(negmax[:], negmax[:], -SC_SCALE)
                    p_t = pp.tile([128, S], F16, tag="p")
                    ssum = stp.tile([128, 1], F32, tag="ss")
                    nc.scalar.activation(
                        out=p_t[:], in_=sc_t[:], func=Act.Exp,
                        bias=negmax[:], scale=SC_SCALE, accum_out=ssum[:])
                    rsum = stp.tile([128, 1], F32, tag="rs")
                    nc.vector.reciprocal(rsum[:], ssum[:])
                    nc.scalar.mul(p_t[:], p_t[:], rsum[:])
                    p_tiles[tt] = p_t

                # tt2's transposes drain now; tt3's go inside the c-A loop
                tr_pending.extend((2, st) for st in range(ST))
                emit_tr(ST)
                tr_pending.extend((3, st) for st in range(ST))

                # --- cT = enc_nat.T @ pT -> [k, t] f16.  kt0-3 split into
                # column sub-ranges of ONE psum group each: cols 0:384 only
                # need tt0-2 of pT, hiding the tt3 softmax+transpose tail;
                # cols 384:512 (tt3) follow once its transposes land ---
                ct_t = ctp.tile([128, HT, T], F16, tag="ct")
                cpss = {}
                for kt in range(4):
                    cps = psmm.tile([128, T], F32, tag="mm", name=f"cps{kt}")
                    for st in range(ST):
                        nc.tensor.matmul(
                            cps[:, 0:384],
                            en_t[:, st, kt * 128:(kt + 1) * 128],
                            pt_t[:, st, 0:384],
                            start=(st == 0), stop=False,
                            skip_group_check=True)
                    emit_tr(2)
                    cpss[kt] = cps
                for kt in range(4):
                    cps = cpss[kt]
                    for st in range(ST):
                        nc.tensor.matmul(
                            cps[:, 384:512],
                            en_t[:, st, kt * 128:(kt + 1) * 128],
                            pt_t[:, st, 384:512],
                            start=False, stop=(st == ST - 1),
                            skip_group_check=True)
                    nc.scalar.copy(ct_t[:, kt, :], cps[:])
                for kt in range(4, HT):
                    cps = psmm.tile([128, T], F32, tag="mm")
                    for st in range(ST):
                        nc.tensor.matmul(
                            cps[:],
                            en_t[:, st, kt * 128:(kt + 1) * 128],
                            pt_t[:, st, :],
                            start=(st == 0), stop=(st == ST - 1))
                    nc.scalar.copy(ct_t[:, kt, :], cps[:])

                # --- out = tanh(cT.T @ WcT + qT.T @ WqT [+ b]) ---
                for tt in range(TT):
                    for hc in range(2):
                        ops = psmm.tile([128, 512], F32, tag="mm")
                        # q-part first: gives tail cT evictions extra slack
                        for ht in range(HT):
                            nc.tensor.matmul(
                                ops[:],
                                qh_t[:, ht, tt * 128:(tt + 1) * 128],
                                wo_t[:, HT + ht, hc * 512:(hc + 1) * 512],
                                start=(ht == 0), stop=False)
                        for kt in range(HT):
                            nc.tensor.matmul(
                                ops[:],
                                ct_t[:, kt, tt * 128:(tt + 1) * 128],
                                wo_t[:, kt, hc * 512:(hc + 1) * 512],
                                start=False, stop=(kt == HT - 1))
                        ost = op.tile([128, 512], F32, tag="os")
                        if has_bias:
                            nc.vector.tensor_add(
                                ost[:], ops[:],
                                bias_t[:, hc * 512:(hc + 1) * 512])
                            nc.scalar.activation(
                                out=ost[:], in_=ost[:], func=Act.Tanh)
                        else:
                            nc.scalar.activation(
                                out=ost[:], in_=ops[:], func=Act.Tanh)
                        nc.sync.dma_start(
                            out_d[b, tt * 128:(tt + 1) * 128,
                                  hc * 512:(hc + 1) * 512],
                            ost[:])

    nc.compile()
    return nc


def _get_nc(has_bias):
    key = ("nc", has_bias)
    if key not in _CACHE:
        _CACHE[key] = _build(has_bias)
    return _CACHE[key]


def _split16(x):
    hi = x.astype(np.float16)
    lo = (x - hi.astype(np.float32)).astype(np.float32)
    return hi, lo


def _f8(x, scale):
    return (np.asarray(x, np.float32) * np.float32(scale)).astype(
        ml_dtypes.float8_e4m3)


def kernel(query, encoder_outputs, src_lengths, W_in, W_out, b_out):
    query = np.asarray(query, np.float32)
    enc = np.asarray(encoder_outputs, np.float32)
    W_in = np.asarray(W_in, np.float32)
    W_out = np.asarray(W_out, np.float32)
    b_out = np.asarray(b_out, np.float32)

    # host-side layout prep (transposes + fp16 hi/lo splits + 2^k scaling)
    qT = np.ascontiguousarray(query.transpose(0, 2, 1))        # [B, H, T]
    qh, ql = _split16(qT)
    encT = np.ascontiguousarray(enc.transpose(1, 2, 0))        # [B, H, S]
    ehf, el = _split16(encT)
    eh = (ehf.astype(np.float32) * 2.0 ** 5).astype(np.float16)
    en = np.ascontiguousarray(enc.transpose(1, 0, 2)).astype(np.float16)
    whf, wlf = _split16(np.ascontiguousarray(W_in.T))          # [H(h), H(i)]
    wh = (whf.astype(np.float32) * 2.0 ** 13).astype(np.float16)
    wo = np.ascontiguousarray(W_out.T).astype(np.float16)      # [2H, H]

    has_bias = bool(np.any(b_out))
    common = {
        "wh": wh, "wo": wo,
        "wh8": _f8(whf.astype(np.float32), 2.0 ** 4),
        "wl8": _f8(wlf, 2.0 ** 13),
    }
    if has_bias:
        common["bias"] = np.ascontiguousarray(
            np.broadcast_to(b_out[None, :], (128, H)), np.float32)

    in_maps = []
    for c in range(NCORES):
        sl = slice(c * BL, (c + 1) * BL)
        m = {
            "qh": np.ascontiguousarray(qh[sl]),
            "eh": np.ascontiguousarray(eh[sl]),
            "en": np.ascontiguousarray(en[sl]),
            "qh8": _f8(qh[sl].astype(np.float32), 1.0),
            "ql8": _f8(ql[sl], 2.0 ** 9),
            "eh8": _f8(ehf[sl].astype(np.float32), 2.0 ** 5),
            "el8": _f8(el[sl], 2.0 ** 13),
            **common,
        }
        in_maps.append(m)

    nc = _get_nc(has_bias)
    trace = bool(int(os.environ.get("KERNEL_TRACE", "0")))
    res = run_bass_kernel_spmd(nc, in_maps, core_ids=list(range(NCORES)),
                               trace=trace)
    if trace:
        _CACHE["last_exec_time_ns"] = res.exec_time_ns
        _CACHE["last_results"] = res
    out = np.concatenate([r["out"] for r in res.results], axis=0)
    return out
